# revision 46
# baseline (speedup 1.0000x reference)
"""Trainium2 Bass kernel for all-pairs log-polar repulsion (gnn_message_passing).

Math: the reference's log-space distance chain collapses in linear space:
  exp(-ld) = 1/sqrt(dx^2+dy^2)  with x = r*(cos t + EPS*sign(cos t)), etc.

Nodes are globally sorted by theta on the host; row-sharded over 8 cores
(512 sorted query rows each). Each core streams 32 j-chunks of 128 nodes;
per chunk computes a [128j x 512i] tile and reduces over j with PE matmuls:
  out0 = sum_j s_j*g_ij                    \
  out1 = sum_j s_j*(ell_j - m)*g_ij         } one 3-row matmul (w3 weights)
  out2 = sum_j s_j*(th_j - pi)*g_ij        /
  outq = -tau*sum_j s_j*g_ij*[th_j-th_i>=pi] + tau*sum_j s_j*g_ij*[th_j-th_i<-pi]
Host assembles (m = mean ell):
  F_ell = s_i*(out1 - (ell_i - m)*out0)
  F_th  = s_i*(out2 - (th_i - pi)*out0 + outq)
then unsorts rows back to the original node order.

The P indicator [th_j-th_i >= pi] can only fire when th_j >= pi; M
[th_j-th_i < -pi] only when th_j < pi. With theta-sorted chunks most chunks
are class-pure, so only ONE indicator op is emitted per chunk (both are
emitted where the per-core column layout makes the class ambiguous - always
correct, just more work). The cutoff mask is applied on f = 1/sqrt(d2)
directly ([f >= phi^-2] == [d2 <= phi^4]).

Per-core layout: local cols 0..3 hold the core's own diagonal chunks (self
pairs zeroed by affine_select, base -128c); cols 4..31 the remaining global
chunks ascending. x/y/(pi-theta) rows are broadcast on-device by PE matmul;
the (pi - theta_i) broadcast stays in PSUM so DVE indicator ops read one
operand from PSUM and one from SBUF (avoids the SBUF port conflict that
makes two-SBUF-input DVE ops run ~2x slower).

Engines per chunk: ACT sqx/sqy/f; Pool d2 (+4 one-time diag affine_selects);
DVE g + 1-2 indicator ops; PE 2-3 matmuls into f32 PSUM (f16 inputs).
"""

import sys

sys.path.insert(0, "/opt/trn_rl_repo")

from contextlib import ExitStack

import numpy as np

import concourse.bacc as bacc
import concourse.bass as bass
import concourse.mybir as mybir
import concourse.tile as tile

N = 4096
NCORES = 8
IPC = N // NCORES  # 512 rows per core
NJC = N // 128  # 32 j-chunks of 128
EPS = np.float32(1e-10)
PHI = (1.0 + np.sqrt(5.0)) / 2.0
TAU32 = float(np.float32(2.0 * np.pi))
PI32 = float(np.float32(np.pi))
CUT2 = float(np.float32(PHI**4))  # dist^2 cutoff = phi^4
INVF = float(np.float32(PHI**-2))  # f cutoff = 1/phi^2  ([f>=INVF] == [d2<=CUT2])
D2BIAS = 1e-8  # keeps f finite in f16 on the (weight-zeroed/masked) diagonal

NCOLS = 8 * NJC  # negx | negy | tauthj | negthj | scol | w3 (3 cols per chunk)
NROWS = 3 * IPC  # x | y | pi - theta rows

_cache = {}


def _col_layouts():
    """col_to_global[k][c] = global chunk held at local col c for core k."""
    layouts = []
    for k in range(NCORES):
        diag = list(range(4 * k, 4 * k + 4))
        others = [g for g in range(NJC) if g not in diag]
        layouts.append(diag + others)
    return layouts


def _emission(chunk_has_p, chunk_has_m):
    """Per local col: (emitP, emitM) = union over cores of that col's chunk
    classes. Emitting an un-needed indicator is harmless (it never fires)."""
    layouts = _col_layouts()
    em = []
    for c in range(NJC):
        p = any(chunk_has_p[layouts[k][c]] for k in range(NCORES))
        mm = any(chunk_has_m[layouts[k][c]] for k in range(NCORES))
        em.append((p, mm))
    return tuple(em)


def _build(emission):
    f32 = mybir.dt.float32
    f16 = mybir.dt.float16
    AF = mybir.ActivationFunctionType
    OP = mybir.AluOpType
    nc = bacc.Bacc()

    d_cols = nc.declare_dram_parameter("cols", [128, NCOLS], f32, isOutput=False)
    d_rows = nc.declare_dram_parameter("rows", [1, NROWS], f32, isOutput=False)
    d_out = nc.declare_dram_parameter("out", [4, IPC], f32, isOutput=True)

    with tile.TileContext(nc) as tc, ExitStack() as ctx:
        const = ctx.enter_context(tc.tile_pool(name="const", bufs=1))
        p_sqx = ctx.enter_context(tc.tile_pool(name="sqx", bufs=3))
        p_sqy = ctx.enter_context(tc.tile_pool(name="sqy", bufs=3))
        p_d2 = ctx.enter_context(tc.tile_pool(name="d2", bufs=3))
        p_f = ctx.enter_context(tc.tile_pool(name="f", bufs=3))
        p_g = ctx.enter_context(tc.tile_pool(name="g", bufs=3))
        p_P = ctx.enter_context(tc.tile_pool(name="P", bufs=3))
        p_M = ctx.enter_context(tc.tile_pool(name="M", bufs=3))
        psum = ctx.enter_context(tc.tile_pool(name="psum", bufs=1, space="PSUM"))

        # rows DMA (tiny, needed first for the broadcasts) on the gpsimd
        # queue; the big cols DMA in parallel on the vector queue
        t_rows = const.tile([1, NROWS], f32)
        nc.gpsimd.dma_start(t_rows[:], d_rows[:])
        t_cols = const.tile([128, NCOLS], f32)
        nc.gpsimd.dma_start(t_cols[:], d_cols[:])

        negx = t_cols[:, 0:NJC]
        negy = t_cols[:, NJC : 2 * NJC]
        tauthj = t_cols[:, 2 * NJC : 3 * NJC]
        negthj = t_cols[:, 3 * NJC : 4 * NJC]
        scol = t_cols[:, 4 * NJC : 5 * NJC]
        w3 = t_cols[:, 5 * NJC : 8 * NJC]

        t_ones = const.tile([1, 128], f32)
        nc.gpsimd.memset(t_ones[:], 1.0)
        t_d2bias = const.tile([128, 1], f32)
        nc.gpsimd.memset(t_d2bias[:], D2BIAS)
        # warm the ACT table with the combined square+abs_rsqrt set so the
        # loop never pays a table swap
        t_actwarm = const.tile([128, 1], f16)
        nc.scalar.activation(
            t_actwarm[:], t_d2bias[:], AF.Abs_reciprocal_sqrt, bias=t_d2bias[:]
        )

        # broadcast x/y/(pi-theta) rows to 128 partitions via PE; keep in PSUM
        pb_x = psum.tile([128, IPC], f32)
        pb_y = psum.tile([128, IPC], f32)
        pb_t = psum.tile([128, IPC], f32)  # pi - theta_i (thrm2), stays in PSUM
        nc.tensor.matmul(pb_x[:], t_ones[:], t_rows[0:1, 0:IPC], start=True, stop=True)
        nc.tensor.matmul(
            pb_y[:], t_ones[:], t_rows[0:1, IPC : 2 * IPC], start=True, stop=True
        )
        nc.tensor.matmul(
            pb_t[:], t_ones[:], t_rows[0:1, 2 * IPC : 3 * IPC], start=True, stop=True
        )

        # derived f16 weights (tiny one-time ops)
        w3h = const.tile([128, 3 * NJC], f16)
        nc.vector.tensor_copy(w3h[:], w3)
        swm = const.tile([128, NJC], f16)  # -tau*s_j for P
        nc.vector.tensor_scalar(swm[:], scol, -TAU32, None, op0=OP.mult)
        swp = const.tile([128, NJC], f16)  # +tau*s_j for M
        nc.vector.tensor_scalar(swp[:], scol, TAU32, None, op0=OP.mult)

        psum3 = psum.tile([3, IPC], f32)
        psumq = psum.tile([1, IPC], f32)

        nq = sum(int(p) + int(mm) for p, mm in emission)
        qi = 0
        d2s = {}
        pend_mm = {}
        SKEW = 1
        MMSKEW = 1  # matmuls batched one stage later (PE bursts -> pstate)
        # software-pipelined: stage A (sqx/sqy/d2) for chunk c, stage B
        # (f/g/mask/indicators) for chunk c-SKEW, stage C (matmuls) for
        # chunk c-SKEW-MMSKEW, so the in-order ACT queue never stalls on
        # Pool's d2 and PE runs in bursts.
        for cc in range(NJC + SKEW + MMSKEW):
            if cc >= SKEW + MMSKEW:
                for mm in pend_mm.pop(cc - SKEW - MMSKEW):
                    mm()
            if cc < NJC:
                c = cc
                sqx = p_sqx.tile([128, IPC], f32)
                nc.scalar.activation(
                    sqx[:], pb_x[:], AF.Square, bias=negx[:, c : c + 1]
                )
                sqy = p_sqy.tile([128, IPC], f32)
                nc.scalar.activation(
                    sqy[:], pb_y[:], AF.Square, bias=negy[:, c : c + 1]
                )
                d2 = p_d2.tile([128, IPC], f32)
                nc.gpsimd.tensor_tensor(d2[:], sqx[:], sqy[:], op=OP.add)
                d2s[c] = d2
            if cc < SKEW or cc >= NJC + SKEW:
                continue
            c = cc - SKEW
            first, last = c == 0, c == NJC - 1
            emitP, emitM = emission[c]
            d2 = d2s.pop(c)
            f = p_f.tile([128, IPC], f16)
            # no d2 bias needed: the diag (d2==0, f=Inf/NaN) is FILLED with 0
            # by affine_select below before any consumer reads it
            nc.scalar.activation(f[:], d2[:], AF.Abs_reciprocal_sqrt)
            if c < 4:  # local diag chunk: zero column i == 128*c + p
                f2 = p_f.tile([128, IPC], f16)
                nc.gpsimd.affine_select(
                    f2[:],
                    f[:],
                    pattern=[[1, IPC]],
                    compare_op=OP.not_equal,
                    fill=0.0,
                    base=-128 * c,
                    channel_multiplier=-1,
                )
                f = f2
            g = p_g.tile([128, IPC], f16)
            nc.vector.scalar_tensor_tensor(
                g[:], f[:], INVF, f[:], op0=OP.is_ge, op1=OP.mult
            )
            mms = []
            if emitP:
                P = p_P.tile([128, IPC], f16)
                nc.vector.scalar_tensor_tensor(
                    P[:], pb_t[:], tauthj[:, c : c + 1], g[:],
                    op0=OP.is_ge, op1=OP.mult,
                )
                mms.append(
                    lambda c=c, P=P, s=qi == 0, e=qi == nq - 1: nc.tensor.matmul(
                        psumq[:], swm[:, c : c + 1], P[:], start=s, stop=e
                    )
                )
                qi += 1
            if emitM:
                M = p_M.tile([128, IPC], f16)
                nc.vector.scalar_tensor_tensor(
                    M[:], pb_t[:], negthj[:, c : c + 1], g[:],
                    op0=OP.is_lt, op1=OP.mult,
                )
                mms.append(
                    lambda c=c, M=M, s=qi == 0, e=qi == nq - 1: nc.tensor.matmul(
                        psumq[:], swp[:, c : c + 1], M[:], start=s, stop=e
                    )
                )
                qi += 1
            mms.append(
                lambda c=c, g=g, s=first, e=last: nc.tensor.matmul(
                    psum3[:], w3h[:, 3 * c : 3 * c + 3], g[:], start=s, stop=e
                )
            )
            pend_mm[c] = mms

        o3 = const.tile([3, IPC], f32)
        nc.vector.tensor_copy(o3[:], psum3[:])
        oq = const.tile([1, IPC], f32)
        nc.vector.tensor_copy(oq[:], psumq[:])
        nc.gpsimd.dma_start(d_out[0:3, :], o3[:])
        nc.gpsimd.dma_start(d_out[3:4, :], oq[:])

    if not nc.is_finalized():
        nc.finalize()
    return nc


def _host_prep(ell, theta, s, frozen):
    f32 = np.float32
    ell = np.asarray(ell, f32)
    theta = np.asarray(theta, f32)
    s = np.asarray(s, f32)
    m = f32(ell.mean())

    sort_idx = np.argsort(theta, kind="stable")
    th_s = theta[sort_idx]
    ell_s = ell[sort_idx]
    s_s = s[sort_idx]

    c = np.cos(th_s).astype(f32)
    sn = np.sin(th_s).astype(f32)
    r = np.exp(ell_s).astype(f32)
    x = (r * (c + EPS * np.sign(c))).astype(f32)
    y = (r * (sn + EPS * np.sign(sn))).astype(f32)

    chunk_th = th_s.reshape(NJC, 128)
    chunk_has_p = [bool((row >= PI32).any()) for row in chunk_th]
    chunk_has_m = [bool((row < PI32).any()) for row in chunk_th]
    emission = _emission(chunk_has_p, chunk_has_m)

    def cols(a):  # [N] -> [128, NJC], global chunk G in column G
        return np.ascontiguousarray(a.reshape(NJC, 128).T)

    xc, yc, thc = cols(x), cols(y), cols(th_s)
    sc, ec = cols(s_s), cols(ell_s)
    w3 = np.stack([sc, sc * (ec - m), sc * (thc - PI32)], axis=2)  # [128, NJC, 3]

    layouts = _col_layouts()
    cols_all = []
    rows_all = []
    for k in range(NCORES):
        perm = layouts[k]
        sl = slice(k * IPC, (k + 1) * IPC)
        payload = np.ascontiguousarray(
            np.concatenate(
                [
                    -xc[:, perm],
                    -yc[:, perm],
                    TAU32 - thc[:, perm],
                    -thc[:, perm],
                    sc[:, perm],
                    w3[:, perm, :].reshape(128, 3 * NJC),
                ],
                axis=1,
            ),
            dtype=f32,
        )
        assert payload.shape == (128, NCOLS)
        cols_all.append(payload)
        rows_all.append(
            np.concatenate([x[sl], y[sl], PI32 - th_s[sl]]).astype(f32)[None, :]
        )
    return cols_all, rows_all, float(m), sort_idx, emission


def _assemble(ell, theta, s, frozen, outs, m, sort_idx):
    ell64 = np.asarray(ell, np.float64)[sort_idx]
    th64 = np.asarray(theta, np.float64)[sort_idx]
    s64 = np.asarray(s, np.float64)[sort_idx]
    nf = 1.0 - np.asarray(frozen, np.float64)[sort_idx]
    Fe = np.empty(N)
    Ft = np.empty(N)
    for k in range(NCORES):
        sl = slice(k * IPC, (k + 1) * IPC)
        o = np.asarray(outs[k], np.float64)
        Fe[sl] = o[1] - (ell64[sl] - m) * o[0]
        Ft[sl] = o[2] - (th64[sl] - np.pi) * o[0] + o[3]
    Fe *= s64 * nf
    Ft *= s64 * nf
    F = np.empty((2, N))
    F[:, sort_idx] = np.stack([Fe, Ft])
    return F.astype(np.float32)


def _get_runner(emission):
    """Build nc once per emission pattern; return a cached jitted executor."""
    key = ("runner", emission)
    if key in _cache:
        return _cache[key]

    import jax
    from jax.sharding import Mesh, PartitionSpec
    from jax.experimental.shard_map import shard_map

    from concourse import bass2jax
    from concourse import mybir as _mybir

    bass2jax.install_neuronx_cc_hook()

    nc = _build(emission)

    in_names = []
    out_names = []
    out_avals = []
    zero_shapes = []
    assert nc.dbg_addr is None
    partition_name = nc.partition_id_tensor.name if nc.partition_id_tensor else None
    for alloc in nc.m.functions[0].allocations:
        if not isinstance(alloc, _mybir.MemoryLocationSet):
            continue
        name = alloc.memorylocations[0].name
        if alloc.kind == "ExternalInput":
            if name != partition_name:
                in_names.append(name)
        elif alloc.kind == "ExternalOutput":
            out_names.append(name)
            shape = tuple(alloc.tensor_shape)
            dtype = _mybir.dt.np(alloc.dtype)
            out_avals.append(jax.core.ShapedArray(shape, dtype))
            zero_shapes.append((shape, dtype))
    n_params = len(in_names)
    n_outs = len(out_avals)
    all_names = in_names + out_names
    if partition_name is not None:
        all_names = all_names + [partition_name]

    donate = tuple(range(n_params, n_params + n_outs))

    def _body(*args):
        operands = list(args)
        if partition_name is not None:
            operands.append(bass2jax.partition_id_tensor())
        outs = bass2jax._bass_exec_p.bind(
            *operands,
            out_avals=tuple(out_avals),
            in_names=tuple(all_names),
            out_names=tuple(out_names),
            lowering_input_output_aliases=(),
            sim_require_finite=True,
            sim_require_nnan=True,
            nc=nc,
        )
        return tuple(outs)

    devices = jax.devices()[:NCORES]
    mesh = Mesh(np.asarray(devices), ("core",))
    in_specs = (PartitionSpec("core"),) * (n_params + n_outs)
    out_specs = (PartitionSpec("core"),) * n_outs
    sharded = jax.jit(
        shard_map(
            _body, mesh=mesh, in_specs=in_specs, out_specs=out_specs, check_rep=False
        ),
        donate_argnums=donate,
        keep_unused=True,
    )

    runner = {
        "fn": sharded,
        "in_names": in_names,
        "out_names": out_names,
        "out_avals": out_avals,
        "zero_shapes": zero_shapes,
        "nc": nc,
    }
    _cache[key] = runner
    return runner


def run_device(ell, theta, s, frozen):
    cols_all, rows_all, m, sort_idx, emission = _host_prep(ell, theta, s, frozen)
    runner = _get_runner(emission)
    in_map = {"cols": cols_all, "rows": rows_all}
    concat_in = [np.concatenate(in_map[name], axis=0) for name in runner["in_names"]]
    concat_zeros = [
        np.zeros((NCORES * shape[0],) + tuple(shape[1:]), dtype)
        for shape, dtype in runner["zero_shapes"]
    ]
    out_arrs = runner["fn"](*concat_in, *concat_zeros)
    oi = runner["out_names"].index("out")
    shape = runner["out_avals"][oi].shape
    outs = np.asarray(out_arrs[oi]).reshape(NCORES, *shape)
    return _assemble(ell, theta, s, frozen, outs, m, sort_idx)


def kernel(ell, theta, s, frozen):
    return run_device(ell, theta, s, frozen)


def _get_runner_default():
    """For test harness introspection: runner for the cached emission, if any."""
    for key in _cache:
        if key[0] == "runner":
            return _cache[key]
    return None


# revision 49
# speedup vs baseline: 1.2048x; 1.2048x over previous
"""Trainium2 Bass kernel for all-pairs log-polar repulsion (gnn_message_passing).

Math: the reference's log-space distance chain collapses in linear space:
  exp(-ld) = 1/sqrt(dx^2+dy^2)  with x = r*(cos t + EPS*sign(cos t)), etc.

Nodes are globally sorted by theta on the host; row-sharded over 8 cores
(512 sorted query rows each). Each core streams 32 j-chunks of 128 nodes;
per chunk computes a [128j x 512i] tile and reduces over j with PE matmuls:
  out0 = sum_j s_j*g_ij                    \
  out1 = sum_j s_j*(ell_j - m)*g_ij         } one 3-row matmul (w3 weights)
  out2 = sum_j s_j*(th_j - pi)*g_ij        /
  outq = -tau*sum_j s_j*g_ij*[th_j-th_i>=pi] + tau*sum_j s_j*g_ij*[th_j-th_i<-pi]
Host assembles (m = mean ell):
  F_ell = s_i*(out1 - (ell_i - m)*out0)
  F_th  = s_i*(out2 - (th_i - pi)*out0 + outq)
then unsorts rows back to the original node order.

The P indicator [th_j-th_i >= pi] can only fire when th_j >= pi; M
[th_j-th_i < -pi] only when th_j < pi. With theta-sorted chunks most chunks
are class-pure, so only ONE indicator op is emitted per chunk (both are
emitted where the per-core column layout makes the class ambiguous - always
correct, just more work). The cutoff mask is applied on f = 1/sqrt(d2)
directly ([f >= phi^-2] == [d2 <= phi^4]).

Per-core layout: local cols 0..3 hold the core's own diagonal chunks (self
pairs zeroed by affine_select, base -128c); cols 4..31 the remaining global
chunks ascending. x/y/(pi-theta) rows are broadcast on-device by PE matmul;
the (pi - theta_i) broadcast stays in PSUM so DVE indicator ops read one
operand from PSUM and one from SBUF (avoids the SBUF port conflict that
makes two-SBUF-input DVE ops run ~2x slower).

Engines per chunk: ACT sqx/sqy/f; Pool d2 (+4 one-time diag affine_selects);
DVE g + 1-2 indicator ops; PE 2-3 matmuls into f32 PSUM (f16 inputs).
"""

import sys

sys.path.insert(0, "/opt/trn_rl_repo")

from contextlib import ExitStack

import numpy as np

import concourse.bacc as bacc
import concourse.bass as bass
import concourse.mybir as mybir
import concourse.tile as tile

N = 4096
NCORES = 8
IPC = N // NCORES  # 512 rows per core
NJC = N // 128  # 32 j-chunks of 128
EPS = np.float32(1e-10)
PHI = (1.0 + np.sqrt(5.0)) / 2.0
TAU32 = float(np.float32(2.0 * np.pi))
PI32 = float(np.float32(np.pi))
CUT2 = float(np.float32(PHI**4))  # dist^2 cutoff = phi^4
INVF = float(np.float32(PHI**-2))  # f cutoff = 1/phi^2  ([f>=INVF] == [d2<=CUT2])
D2BIAS = 1e-8  # keeps f finite in f16 on the (weight-zeroed/masked) diagonal

NCOLS = 8 * NJC  # negx | negy | tauthj | negthj | scol | w3 (3 cols per chunk)
NROWS = 3 * IPC  # x | y | pi - theta rows

_cache = {}


def _col_layouts():
    """col_to_global[k][c] = global chunk held at local col c for core k."""
    layouts = []
    for k in range(NCORES):
        diag = list(range(4 * k, 4 * k + 4))
        others = [g for g in range(NJC) if g not in diag]
        layouts.append(diag + others)
    return layouts


def _emission(chunk_has_p, chunk_has_m):
    """Per local col: (emitP, emitM) = union over cores of that col's chunk
    classes. Emitting an un-needed indicator is harmless (it never fires)."""
    layouts = _col_layouts()
    em = []
    for c in range(NJC):
        p = any(chunk_has_p[layouts[k][c]] for k in range(NCORES))
        mm = any(chunk_has_m[layouts[k][c]] for k in range(NCORES))
        em.append((p, mm))
    return tuple(em)


def _build(emission):
    f32 = mybir.dt.float32
    f16 = mybir.dt.float16
    AF = mybir.ActivationFunctionType
    OP = mybir.AluOpType
    nc = bacc.Bacc()

    d_cols = nc.declare_dram_parameter("cols", [128, NCOLS], f32, isOutput=False)
    d_rows = nc.declare_dram_parameter("rows", [1, NROWS], f32, isOutput=False)
    d_out = nc.declare_dram_parameter("out", [4, IPC], f32, isOutput=True)

    with tile.TileContext(nc) as tc, ExitStack() as ctx:
        const = ctx.enter_context(tc.tile_pool(name="const", bufs=1))
        p_sqx = ctx.enter_context(tc.tile_pool(name="sqx", bufs=3))
        p_sqy = ctx.enter_context(tc.tile_pool(name="sqy", bufs=3))
        p_d2 = ctx.enter_context(tc.tile_pool(name="d2", bufs=3))
        p_f = ctx.enter_context(tc.tile_pool(name="f", bufs=3))
        p_g = ctx.enter_context(tc.tile_pool(name="g", bufs=3))
        p_P = ctx.enter_context(tc.tile_pool(name="P", bufs=3))
        p_M = ctx.enter_context(tc.tile_pool(name="M", bufs=3))
        psum = ctx.enter_context(tc.tile_pool(name="psum", bufs=1, space="PSUM"))

        # rows DMA (tiny, needed first for the broadcasts) on the gpsimd
        # queue; the big cols DMA in parallel on the vector queue
        t_rows = const.tile([1, NROWS], f32)
        nc.gpsimd.dma_start(t_rows[:], d_rows[:])
        t_cols = const.tile([128, NCOLS], f32)
        nc.gpsimd.dma_start(t_cols[:], d_cols[:])

        negx = t_cols[:, 0:NJC]
        negy = t_cols[:, NJC : 2 * NJC]
        tauthj = t_cols[:, 2 * NJC : 3 * NJC]
        negthj = t_cols[:, 3 * NJC : 4 * NJC]
        scol = t_cols[:, 4 * NJC : 5 * NJC]
        w3 = t_cols[:, 5 * NJC : 8 * NJC]

        t_ones = const.tile([1, 128], f32)
        nc.gpsimd.memset(t_ones[:], 1.0)
        t_d2bias = const.tile([128, 1], f32)
        nc.gpsimd.memset(t_d2bias[:], D2BIAS)
        # warm the ACT table with the combined square+abs_rsqrt set so the
        # loop never pays a table swap
        t_actwarm = const.tile([128, 1], f16)
        nc.scalar.activation(
            t_actwarm[:], t_d2bias[:], AF.Abs_reciprocal_sqrt, bias=t_d2bias[:]
        )

        # broadcast x/y/(pi-theta) rows to 128 partitions via PE; keep in PSUM
        pb_x = psum.tile([128, IPC], f32)
        pb_y = psum.tile([128, IPC], f32)
        pb_t = psum.tile([128, IPC], f32)  # pi - theta_i (thrm2), stays in PSUM
        nc.tensor.matmul(pb_x[:], t_ones[:], t_rows[0:1, 0:IPC], start=True, stop=True)
        nc.tensor.matmul(
            pb_y[:], t_ones[:], t_rows[0:1, IPC : 2 * IPC], start=True, stop=True
        )
        nc.tensor.matmul(
            pb_t[:], t_ones[:], t_rows[0:1, 2 * IPC : 3 * IPC], start=True, stop=True
        )

        # derived f16 weights (tiny one-time ops)
        w3h = const.tile([128, 3 * NJC], f16)
        nc.vector.tensor_copy(w3h[:], w3)
        swm = const.tile([128, NJC], f16)  # -tau*s_j for P
        nc.vector.tensor_scalar(swm[:], scol, -TAU32, None, op0=OP.mult)
        swp = const.tile([128, NJC], f16)  # +tau*s_j for M
        nc.vector.tensor_scalar(swp[:], scol, TAU32, None, op0=OP.mult)

        psum3 = psum.tile([3, IPC], f32)
        psumq = psum.tile([1, IPC], f32)

        nq = sum(int(p) + int(mm) for p, mm in emission)
        qi = 0
        d2s = {}
        pend_mm = {}
        SKEW = 1
        MMSKEW = 1  # matmuls batched one stage later (PE bursts -> pstate)
        # software-pipelined: stage A (sqx/sqy/d2) for chunk c, stage B
        # (f/g/mask/indicators) for chunk c-SKEW, stage C (matmuls) for
        # chunk c-SKEW-MMSKEW, so the in-order ACT queue never stalls on
        # Pool's d2 and PE runs in bursts.
        for cc in range(NJC + SKEW + MMSKEW):
            if cc >= SKEW + MMSKEW:
                for mm in pend_mm.pop(cc - SKEW - MMSKEW):
                    mm()
            if cc < NJC:
                c = cc
                sqx = p_sqx.tile([128, IPC], f32)
                nc.scalar.activation(
                    sqx[:], pb_x[:], AF.Square, bias=negx[:, c : c + 1]
                )
                sqy = p_sqy.tile([128, IPC], f32)
                nc.scalar.activation(
                    sqy[:], pb_y[:], AF.Square, bias=negy[:, c : c + 1]
                )
                d2 = p_d2.tile([128, IPC], f32)
                nc.gpsimd.tensor_tensor(d2[:], sqx[:], sqy[:], op=OP.add)
                d2s[c] = d2
            if cc < SKEW or cc >= NJC + SKEW:
                continue
            c = cc - SKEW
            first, last = c == 0, c == NJC - 1
            emitP, emitM = emission[c]
            d2 = d2s.pop(c)
            f_t = p_f.tile([128, IPC + 64], f16)
            f = f_t[:, 0:IPC]
            # no d2 bias needed: the diag (d2==0, f=Inf/NaN) is FILLED with 0
            # by affine_select below before any consumer reads it
            nc.scalar.activation(f[:], d2[:], AF.Abs_reciprocal_sqrt)
            if c < 4:  # local diag chunk: zero column i == 128*c + p
                f2_t = p_f.tile([128, IPC + 64], f16)
                f2 = f2_t[:, 0:IPC]
                nc.gpsimd.affine_select(
                    f2[:],
                    f[:],
                    pattern=[[1, IPC]],
                    compare_op=OP.not_equal,
                    fill=0.0,
                    base=-128 * c,
                    channel_multiplier=-1,
                )
                f = f2
            g_t = p_g.tile([128, IPC + 64], f16)
            g = g_t[:, 0:IPC]
            nc.vector.scalar_tensor_tensor(
                g[:], f[:], INVF, f[:], op0=OP.is_ge, op1=OP.mult
            )
            mms = []
            if emitP:
                P_t = p_P.tile([128, IPC + 64], f16)
                P = P_t[:, 0:IPC]
                nc.vector.scalar_tensor_tensor(
                    P[:], pb_t[:], tauthj[:, c : c + 1], g[:],
                    op0=OP.is_ge, op1=OP.mult,
                )
                mms.append(
                    lambda c=c, P=P, s=qi == 0, e=qi == nq - 1: nc.tensor.matmul(
                        psumq[:], swm[:, c : c + 1], P[:], start=s, stop=e
                    )
                )
                qi += 1
            if emitM:
                M_t = p_M.tile([128, IPC + 64], f16)
                M = M_t[:, 0:IPC]
                nc.vector.scalar_tensor_tensor(
                    M[:], pb_t[:], negthj[:, c : c + 1], g[:],
                    op0=OP.is_lt, op1=OP.mult,
                )
                mms.append(
                    lambda c=c, M=M, s=qi == 0, e=qi == nq - 1: nc.tensor.matmul(
                        psumq[:], swp[:, c : c + 1], M[:], start=s, stop=e
                    )
                )
                qi += 1
            mms.append(
                lambda c=c, g=g, s=first, e=last: nc.tensor.matmul(
                    psum3[:], w3h[:, 3 * c : 3 * c + 3], g[:], start=s, stop=e
                )
            )
            pend_mm[c] = mms

        o3 = const.tile([3, IPC], f32)
        nc.vector.tensor_copy(o3[:], psum3[:])
        oq = const.tile([1, IPC], f32)
        nc.vector.tensor_copy(oq[:], psumq[:])
        nc.gpsimd.dma_start(d_out[0:3, :], o3[:])
        nc.gpsimd.dma_start(d_out[3:4, :], oq[:])

    if not nc.is_finalized():
        nc.finalize()
    return nc


def _host_prep(ell, theta, s, frozen):
    f32 = np.float32
    ell = np.asarray(ell, f32)
    theta = np.asarray(theta, f32)
    s = np.asarray(s, f32)
    m = f32(ell.mean())

    sort_idx = np.argsort(theta, kind="stable")
    th_s = theta[sort_idx]
    ell_s = ell[sort_idx]
    s_s = s[sort_idx]

    c = np.cos(th_s).astype(f32)
    sn = np.sin(th_s).astype(f32)
    r = np.exp(ell_s).astype(f32)
    x = (r * (c + EPS * np.sign(c))).astype(f32)
    y = (r * (sn + EPS * np.sign(sn))).astype(f32)

    chunk_th = th_s.reshape(NJC, 128)
    chunk_has_p = [bool((row >= PI32).any()) for row in chunk_th]
    chunk_has_m = [bool((row < PI32).any()) for row in chunk_th]
    emission = _emission(chunk_has_p, chunk_has_m)

    def cols(a):  # [N] -> [128, NJC], global chunk G in column G
        return np.ascontiguousarray(a.reshape(NJC, 128).T)

    xc, yc, thc = cols(x), cols(y), cols(th_s)
    sc, ec = cols(s_s), cols(ell_s)
    w3 = np.stack([sc, sc * (ec - m), sc * (thc - PI32)], axis=2)  # [128, NJC, 3]

    layouts = _col_layouts()
    cols_all = []
    rows_all = []
    for k in range(NCORES):
        perm = layouts[k]
        sl = slice(k * IPC, (k + 1) * IPC)
        payload = np.ascontiguousarray(
            np.concatenate(
                [
                    -xc[:, perm],
                    -yc[:, perm],
                    TAU32 - thc[:, perm],
                    -thc[:, perm],
                    sc[:, perm],
                    w3[:, perm, :].reshape(128, 3 * NJC),
                ],
                axis=1,
            ),
            dtype=f32,
        )
        assert payload.shape == (128, NCOLS)
        cols_all.append(payload)
        rows_all.append(
            np.concatenate([x[sl], y[sl], PI32 - th_s[sl]]).astype(f32)[None, :]
        )
    return cols_all, rows_all, float(m), sort_idx, emission


def _assemble(ell, theta, s, frozen, outs, m, sort_idx):
    ell64 = np.asarray(ell, np.float64)[sort_idx]
    th64 = np.asarray(theta, np.float64)[sort_idx]
    s64 = np.asarray(s, np.float64)[sort_idx]
    nf = 1.0 - np.asarray(frozen, np.float64)[sort_idx]
    Fe = np.empty(N)
    Ft = np.empty(N)
    for k in range(NCORES):
        sl = slice(k * IPC, (k + 1) * IPC)
        o = np.asarray(outs[k], np.float64)
        Fe[sl] = o[1] - (ell64[sl] - m) * o[0]
        Ft[sl] = o[2] - (th64[sl] - np.pi) * o[0] + o[3]
    Fe *= s64 * nf
    Ft *= s64 * nf
    F = np.empty((2, N))
    F[:, sort_idx] = np.stack([Fe, Ft])
    return F.astype(np.float32)


def _get_runner(emission):
    """Build nc once per emission pattern; return a cached jitted executor."""
    key = ("runner", emission)
    if key in _cache:
        return _cache[key]

    import jax
    from jax.sharding import Mesh, PartitionSpec
    from jax.experimental.shard_map import shard_map

    from concourse import bass2jax
    from concourse import mybir as _mybir

    bass2jax.install_neuronx_cc_hook()

    nc = _build(emission)

    in_names = []
    out_names = []
    out_avals = []
    zero_shapes = []
    assert nc.dbg_addr is None
    partition_name = nc.partition_id_tensor.name if nc.partition_id_tensor else None
    for alloc in nc.m.functions[0].allocations:
        if not isinstance(alloc, _mybir.MemoryLocationSet):
            continue
        name = alloc.memorylocations[0].name
        if alloc.kind == "ExternalInput":
            if name != partition_name:
                in_names.append(name)
        elif alloc.kind == "ExternalOutput":
            out_names.append(name)
            shape = tuple(alloc.tensor_shape)
            dtype = _mybir.dt.np(alloc.dtype)
            out_avals.append(jax.core.ShapedArray(shape, dtype))
            zero_shapes.append((shape, dtype))
    n_params = len(in_names)
    n_outs = len(out_avals)
    all_names = in_names + out_names
    if partition_name is not None:
        all_names = all_names + [partition_name]

    donate = tuple(range(n_params, n_params + n_outs))

    def _body(*args):
        operands = list(args)
        if partition_name is not None:
            operands.append(bass2jax.partition_id_tensor())
        outs = bass2jax._bass_exec_p.bind(
            *operands,
            out_avals=tuple(out_avals),
            in_names=tuple(all_names),
            out_names=tuple(out_names),
            lowering_input_output_aliases=(),
            sim_require_finite=True,
            sim_require_nnan=True,
            nc=nc,
        )
        return tuple(outs)

    devices = jax.devices()[:NCORES]
    mesh = Mesh(np.asarray(devices), ("core",))
    in_specs = (PartitionSpec("core"),) * (n_params + n_outs)
    out_specs = (PartitionSpec("core"),) * n_outs
    sharded = jax.jit(
        shard_map(
            _body, mesh=mesh, in_specs=in_specs, out_specs=out_specs, check_rep=False
        ),
        donate_argnums=donate,
        keep_unused=True,
    )

    runner = {
        "fn": sharded,
        "in_names": in_names,
        "out_names": out_names,
        "out_avals": out_avals,
        "zero_shapes": zero_shapes,
        "nc": nc,
    }
    _cache[key] = runner
    return runner


def run_device(ell, theta, s, frozen):
    cols_all, rows_all, m, sort_idx, emission = _host_prep(ell, theta, s, frozen)
    runner = _get_runner(emission)
    in_map = {"cols": cols_all, "rows": rows_all}
    concat_in = [np.concatenate(in_map[name], axis=0) for name in runner["in_names"]]
    concat_zeros = [
        np.zeros((NCORES * shape[0],) + tuple(shape[1:]), dtype)
        for shape, dtype in runner["zero_shapes"]
    ]
    out_arrs = runner["fn"](*concat_in, *concat_zeros)
    oi = runner["out_names"].index("out")
    shape = runner["out_avals"][oi].shape
    outs = np.asarray(out_arrs[oi]).reshape(NCORES, *shape)
    return _assemble(ell, theta, s, frozen, outs, m, sort_idx)


def kernel(ell, theta, s, frozen):
    return run_device(ell, theta, s, frozen)


def _get_runner_default():
    """For test harness introspection: runner for the cached emission, if any."""
    for key in _cache:
        if key[0] == "runner":
            return _cache[key]
    return None
